# revision 43
# baseline (speedup 1.0000x reference)
"""Bidirectional LSTM on 8 Trainium2 NeuronCores (SPMD, Bass/Tile).

Problem:  x[512,64,512] -> BiLSTM(H=512) -> Linear(1024->512) -> out[512,64,512]

Sharding: batch 4-way x direction 2-way.
  core c   (c in 0..3): forward  LSTM, batch rows [c*128, (c+1)*128)
  core c+4            : backward LSTM, same rows (host passes x time-reversed)

v2: the recurrent GEMM runs in fp8e4m3 with DoubleRow (2 fp8 weights/PE cell,
K=256 per matmul -> 8 matmuls/step instead of 16). Numerics: h is ~4x smaller
than x in gate magnitude, so fp8 noise on the h-path lands at ~8e-3 rel err
(sim), vs 4e-2 if the x-path went fp8. Scale plumbing keeps one PSUM scale:
  gates_psum = x @ (64 W_ih)  +  h @ (64 W_hh)             [both x64]
  act = func(gates_psum * (1/64))                           [free ACT scale]
(64*W_hh in e4m3 is fine: |64 W_hh| <= 2.9, well inside the format; h is
stored unscaled — its sub-0.0156 values hit e4m3 subnormals but contribute
negligibly.) Cell state + elementwise chain run in bf16 (2x DVE modes); ACT
does the sigmoid/tanh with the /64 descale folded in.

Per-core device program (identical NEFF on all 8 cores, different data):
  for t in 0..63:
    g  = x_t @ W64_ih.T (bf16) + h_{t-1} @ 64W_hh.T (fp8 DoubleRow)   PE
    gg = tanh(g/64); i,f,o = sigmoid(g/64)        ACT, bf16 out
    c  = f*c + i*gg ; h = o*tanh(c)               DVE bf16
    hsT16[:,:,t*128:+128] = h.T (PE transpose + DVE copy), hsT8 = fp8 (ACT)
  outT = w_lin_half @ hsT16                       PE, streamed over 8192 tokens

Host: prepares transposed/scaled operands, gathers the 8 partial outputs,
adds forward+backward partials + b_lin in fp32, restores [B,T,O] layout.
"""

import os
import sys

import numpy as np
import ml_dtypes

sys.path.insert(0, "/opt/trn_rl_repo")

import concourse.bass as bass  # noqa: E402
import concourse.tile as tile  # noqa: E402
from concourse import bacc, mybir  # noqa: E402
from concourse.bass_utils import run_bass_kernel_spmd  # noqa: E402

BF16 = ml_dtypes.bfloat16
FP8 = ml_dtypes.float8_e4m3
B, T, I, H, O = 512, 64, 512, 512, 512
BL = 128        # batch rows per core
G4 = 4 * H      # 2048 gate rows
NTOK = T * BL   # 8192 tokens per core
AF = mybir.ActivationFunctionType
DR = mybir.MatmulPerfMode.DoubleRow

# our gate order [g, i, f, o]; pytorch rows are [i, f, g, o]
_PERM = np.concatenate([
    np.arange(2 * H, 3 * H),   # g (cell candidate)
    np.arange(0, H),           # i
    np.arange(H, 2 * H),       # f
    np.arange(3 * H, 4 * H),   # o
])
IG, II, IF, IO = 0, 1, 2, 3

_PROGRAM = None
_LAST_RESULTS = None


def _build_program():
    dt = mybir.dt
    nc = bacc.Bacc("TRN2", target_bir_lowering=False, debug=False)

    xT_d = nc.dram_tensor("xT", [T, 128, 4, BL], dt.bfloat16, kind="ExternalInput")
    xT8_d = nc.dram_tensor("xT8", [T, 128, 4, BL], dt.float8e4, kind="ExternalInput")
    # bf16 input weights for gates g,i,f only; the o-gate x-GEMM runs fp8
    # DoubleRow (sim: 1.27e-2 rel err, budget 2e-2)
    wih_d = nc.dram_tensor("wihT", [128, 4, 3 * H], dt.bfloat16, kind="ExternalInput")
    wih8o_d = nc.dram_tensor("wih8o", [128, 4, H], dt.float8e4, kind="ExternalInput")
    whh_d = nc.dram_tensor("whhT", [128, 4, G4], dt.float8e4, kind="ExternalInput")
    bbc_d = nc.dram_tensor("bbc", [1, G4], dt.bfloat16, kind="ExternalInput")
    wlin_d = nc.dram_tensor("wlinT", [128, 4, O], dt.bfloat16, kind="ExternalInput")
    ident_d = nc.dram_tensor("ident", [128, 128], dt.bfloat16, kind="ExternalInput")
    outT_d = nc.dram_tensor("outT", [4, 128, NTOK], dt.float32, kind="ExternalOutput")

    xap = xT_d.ap()
    xap8 = xT8_d.ap()
    oap = outT_d.ap()

    with tile.TileContext(nc) as tc:
        with (
            tc.tile_pool(name="const", bufs=1) as constp,
            tc.tile_pool(name="hs", bufs=1) as hsp,
        ):
            # ACT table warmup: load the sigmoid/tanh spline set during DMAs
            warm = constp.tile([128, 128], dt.bfloat16)
            nc.gpsimd.memset(warm[:], 0.0)
            warm2 = constp.tile([128, 1], dt.float32)
            nc.scalar.activation(warm2[:], warm[:, :1], AF.Sigmoid)
            # ones row: stationary for the K=1 bias outer-product matmuls
            ones = constp.tile([1, 128], dt.bfloat16)
            nc.gpsimd.memset(ones[:], 1.0)

            # ordered by first use; x loads go on gpsimd's DMA queue so the
            # first timesteps don't queue behind weights.
            # wih in per-(k, gate) 128KB chunks, alternating queues, in the
            # exact order the t=0 x matmuls consume them
            wih = constp.tile([128, 4, 3 * H], dt.bfloat16)
            q = 0
            for k in range(4):
                for n in range(3):
                    eng = nc.sync if q % 2 == 0 else nc.scalar
                    eng.dma_start(
                        wih[:, k, bass.ts(n, 512)], wih_d.ap()[:, k, bass.ts(n, 512)]
                    )
                    q += 1
            wih8o = constp.tile([128, 4, H], dt.float8e4)
            nc.sync.dma_start(wih8o[:, 0:2, :], wih8o_d.ap()[:, 0:2, :])
            nc.scalar.dma_start(wih8o[:, 2:4, :], wih8o_d.ap()[:, 2:4, :])
            bbc = constp.tile([1, G4], dt.bfloat16)
            nc.scalar.dma_start(bbc[:], bbc_d[:])
            ident = constp.tile([128, 128], dt.bfloat16)
            nc.gpsimd.dma_start(ident[:], ident_d[:])
            whh = constp.tile([128, 4, G4], dt.float8e4)
            for k in range(4):
                eng = nc.sync if k % 2 == 0 else nc.scalar
                eng.dma_start(whh[:, k, :], whh_d.ap()[:, k, :])
            wlin = constp.tile([128, 4, O], dt.bfloat16)
            # wlin DMA is issued inside the t=0 body, after xT[0]'s, so it
            # doesn't delay the first x matmuls on the gpsimd queue

            # h.T history (stored as 8*h.T): bf16 for the linear,
            # fp8 pair-sliceable for the DoubleRow recurrence
            hsT16 = hsp.tile([128, 4, NTOK], dt.bfloat16, name="hsT16")
            hsT8 = hsp.tile([128, 4, NTOK], dt.float8e4, name="hsT8")

            with (
                tc.tile_pool(name="xin", bufs=6) as xp,
                tc.tile_pool(name="xin8", bufs=6) as xp8,
                tc.tile_pool(name="gates", bufs=5, space="PSUM") as gps,
                tc.tile_pool(name="trps", bufs=1, space="PSUM") as trp,
                tc.tile_pool(name="linps", bufs=2, space="PSUM") as linps,
                tc.tile_pool(name="cell", bufs=3) as cp,
                tc.tile_pool(name="acts", bufs=8) as app,
                tc.tile_pool(name="linsb", bufs=6) as linsb,
            ):
                def emit_transpose(h_halves, t, need8=True):
                    tr = trp.tile([128, 512], dt.bfloat16, tag="tr")
                    for j in range(4):
                        nc.tensor.transpose(
                            tr[:, bass.ts(j, 128)],
                            h_halves[j // 2][:, bass.ts(j % 2, 128)],
                            ident[:],
                        )
                    # recurrence-critical fp8 copies go FIRST (the tile
                    # framework serializes readers of tr in program order)
                    # and on DVE, split in k-pair halves so the first
                    # DoubleRow matmul unblocks as early as possible
                    if need8:
                        # one copy so both k-pairs become ready together and
                        # the scheduler keeps the recurrence matmuls
                        # gate-outer (gate g completes earliest -> shorter
                        # ACT/DVE chain)
                        nc.vector.tensor_copy(hsT8[:, :, bass.ts(t, 128)], tr[:])
                        # bf16 copy (feeds the linear, 5 steps behind) on ACT
                        nc.scalar.copy(hsT16[:, :, bass.ts(t, 128)], tr[:])
                    else:
                        nc.vector.tensor_copy(hsT16[:, :, bass.ts(t, 128)], tr[:])

                def emit_linear_mm(nch, m):
                    # outT[m] partial for token chunk nch (PE-idle filler)
                    ps = linps.tile(
                        [128, 512], dt.float32, tag="lps", name=f"lin{nch}_{m}"
                    )
                    for k in range(4):
                        nc.tensor.matmul(
                            ps[:], wlin[:, k, bass.ts(m, 128)],
                            hsT16[:, k, bass.ts(nch, 512)],
                            start=(k == 0), stop=(k == 3),
                        )
                    return ps

                def emit_linear_out(ps, nch, m):
                    # ACT copy deferred to the next step so it lands AFTER
                    # that step's hsT8 copy in the ACT FIFO (it must not
                    # delay the recurrence-critical copy)
                    ob = linsb.tile(
                        [128, 512], dt.float32, tag="ob", name=f"ob{nch}_{m}"
                    )
                    nc.scalar.copy(ob[:], ps[:])
                    eng = nc.sync if m % 2 == 0 else nc.scalar
                    eng.dma_start(oap[m, :, bass.ts(nch, 512)], ob[:])

                def emit_linear(nch, m):
                    emit_linear_out(emit_linear_mm(nch, m), nch, m)

                # PE warm-up during the initial weight DMAs: keeps the HAM
                # activity monitor from starting the real matmuls at the
                # throttled 1.2 GHz clock (zeros x zeros into scratch PSUM)
                wps = [
                    gps.tile([128, 512], dt.float32, tag="g", name=f"warm{i}")
                    for i in range(2)
                ]
                for w in range(8):
                    nc.tensor.matmul(
                        wps[w % 2][:, :128], warm[:], warm[:],
                        start=True, stop=True,
                    )

                c_prev = None
                h_prev = None
                lin_pending = None
                for t in range(T):
                    xt = xp.tile([128, 4, BL], dt.bfloat16, tag="xt")
                    nc.gpsimd.dma_start(xt[:], xap[t])
                    xt8 = xp8.tile([128, 4, BL], dt.float8e4, tag="xt8")
                    nc.gpsimd.dma_start(xt8[:], xap8[t])
                    if t == 0:
                        nc.gpsimd.dma_start(wlin[:], wlin_d[:])

                    gt = [
                        gps.tile([128, 512], dt.float32, tag="g", name=f"g{t}_{n}")
                        for n in range(4)
                    ]
                    # bias opens each gate's PSUM accumulation group as a K=1
                    # outer product (ones x bias row) — keeps the bias add off
                    # the DVE and out of the recurrence-critical chain: the
                    # activations then read PSUM directly
                    for n in range(4):
                        nc.tensor.matmul(
                            gt[n][:], ones[:], bbc[:, bass.ts(n, 512)],
                            start=True, stop=False,
                        )
                    for k in range(4):
                        for n in range(3):
                            nc.tensor.matmul(
                                gt[n][:], xt[:, k, :], wih[:, k, bass.ts(n, 512)],
                                start=False, stop=(t == 0 and k == 3),
                            )
                    # o-gate x contribution in fp8 DoubleRow (x8 stationary)
                    for j in range(2):
                        nc.tensor.matmul(
                            gt[IO][:],
                            xt8[:, 2 * j:2 * j + 2, :],
                            wih8o[:, 2 * j:2 * j + 2, :],
                            start=False, stop=(t == 0 and j == 1),
                            perf_mode=DR,
                        )
                    if t > 0:
                        # transpose of h_{t-1} goes here: the x matmuls above
                        # cover step t-1's ACT/DVE chain latency
                        emit_transpose(h_prev, t - 1)
                        if lin_pending is not None:
                            emit_linear_out(*lin_pending)
                            lin_pending = None
                        if t >= 5:
                            # linear filler sits between the transposes and
                            # the recurrence matmuls: it covers the hsT8-copy
                            # wait so the PE never idles there
                            p = t - 5
                            lin_pending = (
                                emit_linear_mm(p // 4, p % 4), p // 4, p % 4
                            )
                        # fp8 DoubleRow recurrence: 2 k-pair matmuls per gate,
                        # gate-outer so each gate's PSUM tile completes early
                        for n in range(4):
                            for j in range(2):
                                nc.tensor.matmul(
                                    gt[n][:],
                                    hsT8[:, 2 * j:2 * j + 2, bass.ts(t - 1, 128)],
                                    whh[:, 2 * j:2 * j + 2, bass.ts(n, 512)],
                                    start=False, stop=(j == 1),
                                    perf_mode=DR,
                                )

                    # activations read the gate PSUM directly (bias is already
                    # accumulated there); /64 descale rides the free ACT scale
                    acts = {}
                    for n, fn in ((IG, AF.Tanh), (II, AF.Sigmoid), (IF, AF.Sigmoid)):
                        a = app.tile([128, 512], dt.bfloat16, tag="act", name=f"act{t}_{n}")
                        nc.scalar.activation(a[:], gt[n][:], fn, scale=1.0 / 64.0)
                        acts[n] = a
                    tg, i_s, f_s = acts[IG], acts[II], acts[IF]

                    c_new = cp.tile([128, 512], dt.bfloat16, tag="c")
                    if t == 0:
                        o_s = app.tile([128, 512], dt.bfloat16, tag="act", name=f"act{t}_o")
                        nc.scalar.activation(o_s[:], gt[IO][:], AF.Sigmoid, scale=1.0 / 64.0)
                        nc.vector.tensor_mul(c_new[:], i_s[:], tg[:])
                    else:
                        ig = cp.tile([128, 512], dt.bfloat16, tag="ig")
                        nc.vector.tensor_mul(ig[:], i_s[:], tg[:])
                        o_s = app.tile([128, 512], dt.bfloat16, tag="act", name=f"act{t}_o")
                        nc.scalar.activation(o_s[:], gt[IO][:], AF.Sigmoid, scale=1.0 / 64.0)
                        fc = cp.tile([128, 512], dt.bfloat16, tag="fc")
                        nc.vector.tensor_mul(fc[:], f_s[:], c_prev[:])
                        nc.vector.tensor_add(c_new[:], ig[:], fc[:])
                    c_prev = c_new

                    # tanh(c) -> h in halves: the first half unblocks the PE
                    # transposes earlier than a monolithic tail would
                    h_halves = []
                    for hh in range(2):
                        sl = bass.ts(hh, 256)
                        tch = app.tile(
                            [128, 256], dt.bfloat16, tag=f"tch{hh}", name=f"tch{t}_{hh}"
                        )
                        nc.scalar.activation(tch[:], c_new[:, sl], AF.Tanh)
                        hb = cp.tile(
                            [128, 256], dt.bfloat16, tag=f"h{hh}", name=f"h{t}_{hh}"
                        )
                        nc.vector.tensor_mul(hb[:], o_s[:, sl], tch[:])
                        h_halves.append(hb)
                    h_prev = h_halves

                emit_linear_out(*lin_pending)
                emit_linear(14, 3)
                # last token chunk split: tokens 60-62 (N=384) fill the PE
                # while step 63's ACT/DVE chain finishes; token 63 (N=128)
                # must wait for the final transpose
                # tail linear parts use the gates pool (free after step 63):
                # 4 tiles in flight, so the 16 matmuls stream without waiting
                # on per-m output copies
                for m in range(4):
                    ps = gps.tile([128, 384], dt.float32, tag="g", name=f"linA_{m}")
                    for k in range(4):
                        nc.tensor.matmul(
                            ps[:], wlin[:, k, bass.ts(m, 128)],
                            hsT16[:, k, 15 * 512:15 * 512 + 384],
                            start=(k == 0), stop=(k == 3),
                        )
                    ob = linsb.tile([128, 384], dt.float32, tag="ob", name=f"obA_{m}")
                    nc.vector.tensor_copy(ob[:], ps[:])
                    eng = nc.sync if m % 2 == 0 else nc.scalar
                    eng.dma_start(oap[m, :, 15 * 512:15 * 512 + 384], ob[:])
                emit_transpose(h_prev, T - 1, need8=False)
                for m in range(4):
                    ps = gps.tile([128, 128], dt.float32, tag="g", name=f"linB_{m}")
                    for k in range(4):
                        nc.tensor.matmul(
                            ps[:], wlin[:, k, bass.ts(m, 128)],
                            hsT16[:, k, 15 * 512 + 384:NTOK],
                            start=(k == 0), stop=(k == 3),
                        )
                    ob = linsb.tile([128, 128], dt.float32, tag="ob", name=f"obB_{m}")
                    nc.vector.tensor_copy(ob[:], ps[:])
                    eng = nc.sync if m % 2 == 0 else nc.scalar
                    eng.dma_start(oap[m, :, 15 * 512 + 384:NTOK], ob[:])


    nc.compile()
    return nc


def _get_program():
    global _PROGRAM
    if _PROGRAM is None:
        _PROGRAM = _build_program()
    return _PROGRAM


def _prep_core_inputs(xc, w_ih, w_hh, b, w_lin_half, backward):
    # xc: [BL, T, I] fp32 batch chunk
    if backward:
        xc = xc[:, ::-1, :]
    # [T, i_k(128) partitions, k(4), b(128)]
    xTf = np.ascontiguousarray(
        xc.transpose(1, 2, 0).reshape(T, 4, 128, BL).transpose(0, 2, 1, 3)
    )
    xT = xTf.astype(BF16)
    xT8 = xTf.astype(FP8)
    wp = 64.0 * w_ih[_PERM]  # [4H, I] in [g,i,f,o] order, pre-scaled
    wihT = np.ascontiguousarray(
        wp[: 3 * H].T.reshape(4, 128, 3 * H).transpose(1, 0, 2)
    ).astype(BF16)
    wih8o = np.ascontiguousarray(
        wp[3 * H:].T.reshape(4, 128, H).transpose(1, 0, 2)
    ).astype(FP8)
    whhT = np.ascontiguousarray(
        (64.0 * w_hh[_PERM]).T.reshape(4, 128, G4).transpose(1, 0, 2)
    ).astype(FP8)
    bbc = np.ascontiguousarray((64.0 * b[_PERM])[None, :].astype(BF16))
    wlinT = np.ascontiguousarray(
        w_lin_half.T.reshape(4, 128, O).transpose(1, 0, 2)
    ).astype(BF16)
    ident = np.eye(128, dtype=BF16)
    return dict(
        xT=xT, xT8=xT8, wihT=wihT, wih8o=wih8o, whhT=whhT, bbc=bbc,
        wlinT=wlinT, ident=ident,
    )


def kernel(x, w_ih_f, w_hh_f, b_f, w_ih_b, w_hh_b, b_b, w_lin, b_lin):
    global _LAST_RESULTS
    x = np.asarray(x, np.float32)
    w_ih_f = np.asarray(w_ih_f, np.float32)
    w_hh_f = np.asarray(w_hh_f, np.float32)
    b_f = np.asarray(b_f, np.float32)
    w_ih_b = np.asarray(w_ih_b, np.float32)
    w_hh_b = np.asarray(w_hh_b, np.float32)
    b_b = np.asarray(b_b, np.float32)
    w_lin = np.asarray(w_lin, np.float32)
    b_lin = np.asarray(b_lin, np.float32)

    nc = _get_program()
    in_maps = []
    for core in range(8):
        cidx = core % 4
        xc = x[cidx * BL:(cidx + 1) * BL]
        if core < 4:
            in_maps.append(
                _prep_core_inputs(xc, w_ih_f, w_hh_f, b_f, w_lin[:, :H], False)
            )
        else:
            in_maps.append(
                _prep_core_inputs(xc, w_ih_b, w_hh_b, b_b, w_lin[:, H:], True)
            )

    trace = bool(int(os.environ.get("LSTM_TRACE", "0")))
    tcores = os.environ.get("LSTM_TRACE_CORES", "")
    kwargs = {}
    if trace and tcores:
        kwargs["trace_cores"] = [int(c) for c in tcores.split(",")]
    res = run_bass_kernel_spmd(
        nc, in_maps, core_ids=list(range(8)), trace=trace, **kwargs
    )
    _LAST_RESULTS = res

    out = np.empty((B, T, O), np.float32)
    for cidx in range(4):
        pf = np.asarray(res.results[cidx]["outT"], np.float32)
        pb = np.asarray(res.results[cidx + 4]["outT"], np.float32)
        pf = pf.reshape(4, 128, T, BL).transpose(3, 2, 0, 1).reshape(BL, T, O)
        pb = pb.reshape(4, 128, T, BL).transpose(3, 2, 0, 1).reshape(BL, T, O)[:, ::-1]
        out[cidx * BL:(cidx + 1) * BL] = pf + pb + b_lin[None, None, :]
    return out


# revision 52
# speedup vs baseline: 1.0684x; 1.0684x over previous
"""Bidirectional LSTM on 8 Trainium2 NeuronCores (SPMD, Bass/Tile).

Problem:  x[512,64,512] -> BiLSTM(H=512) -> Linear(1024->512) -> out[512,64,512]

Sharding: batch 4-way x direction 2-way.
  core c   (c in 0..3): forward  LSTM, batch rows [c*128, (c+1)*128)
  core c+4            : backward LSTM, same rows (host passes x time-reversed)

v2: the recurrent GEMM runs in fp8e4m3 with DoubleRow (2 fp8 weights/PE cell,
K=256 per matmul -> 8 matmuls/step instead of 16). Numerics: h is ~4x smaller
than x in gate magnitude, so fp8 noise on the h-path lands at ~8e-3 rel err
(sim), vs 4e-2 if the x-path went fp8. Scale plumbing keeps one PSUM scale:
  gates_psum = x @ (64 W_ih)  +  h @ (64 W_hh)             [both x64]
  act = func(gates_psum * (1/64))                           [free ACT scale]
(64*W_hh in e4m3 is fine: |64 W_hh| <= 2.9, well inside the format; h is
stored unscaled — its sub-0.0156 values hit e4m3 subnormals but contribute
negligibly.) Cell state + elementwise chain run in bf16 (2x DVE modes); ACT
does the sigmoid/tanh with the /64 descale folded in.

Per-core device program (identical NEFF on all 8 cores, different data):
  for t in 0..63:
    g  = x_t @ W64_ih.T (bf16) + h_{t-1} @ 64W_hh.T (fp8 DoubleRow)   PE
    gg = tanh(g/64); i,f,o = sigmoid(g/64)        ACT, bf16 out
    c  = f*c + i*gg ; h = o*tanh(c)               DVE bf16
    hsT16[:,:,t*128:+128] = h.T (PE transpose + DVE copy), hsT8 = fp8 (ACT)
  outT = w_lin_half @ hsT16                       PE, streamed over 8192 tokens

Host: prepares transposed/scaled operands, gathers the 8 partial outputs,
adds forward+backward partials + b_lin in fp32, restores [B,T,O] layout.
"""

import os
import sys

import numpy as np
import ml_dtypes

sys.path.insert(0, "/opt/trn_rl_repo")

import concourse.bass as bass  # noqa: E402
import concourse.tile as tile  # noqa: E402
from concourse import bacc, mybir  # noqa: E402
from concourse.bass_utils import run_bass_kernel_spmd  # noqa: E402

BF16 = ml_dtypes.bfloat16
FP8 = ml_dtypes.float8_e4m3
B, T, I, H, O = 512, 64, 512, 512, 512
BL = 128        # batch rows per core
G4 = 4 * H      # 2048 gate rows
NTOK = T * BL   # 8192 tokens per core
AF = mybir.ActivationFunctionType
DR = mybir.MatmulPerfMode.DoubleRow

# our gate order [g, i, f, o]; pytorch rows are [i, f, g, o]
_PERM = np.concatenate([
    np.arange(2 * H, 3 * H),   # g (cell candidate)
    np.arange(0, H),           # i
    np.arange(H, 2 * H),       # f
    np.arange(3 * H, 4 * H),   # o
])
IG, II, IF, IO = 0, 1, 2, 3

_PROGRAM = None
_LAST_RESULTS = None


def _build_program():
    dt = mybir.dt
    nc = bacc.Bacc("TRN2", target_bir_lowering=False, debug=False)

    xT_d = nc.dram_tensor("xT", [T, 128, 4, BL], dt.bfloat16, kind="ExternalInput")
    xT8_d = nc.dram_tensor("xT8", [T, 128, 4, BL], dt.float8e4, kind="ExternalInput")
    # bf16 input weights for gates g,i,f only; the o-gate x-GEMM runs fp8
    # DoubleRow (sim: 1.27e-2 rel err, budget 2e-2)
    wih_d = nc.dram_tensor("wihT", [128, 4, 3 * H], dt.bfloat16, kind="ExternalInput")
    wih8o_d = nc.dram_tensor("wih8o", [128, 4, H], dt.float8e4, kind="ExternalInput")
    whh_d = nc.dram_tensor("whhT", [128, 4, G4], dt.float8e4, kind="ExternalInput")
    bbc_d = nc.dram_tensor("bbc", [128, G4], dt.bfloat16, kind="ExternalInput")
    wlin_d = nc.dram_tensor("wlinT", [128, 4, O], dt.bfloat16, kind="ExternalInput")
    ident_d = nc.dram_tensor("ident", [128, 128], dt.bfloat16, kind="ExternalInput")
    outT_d = nc.dram_tensor("outT", [4, 128, NTOK], dt.float32, kind="ExternalOutput")

    xap = xT_d.ap()
    xap8 = xT8_d.ap()
    oap = outT_d.ap()

    with tile.TileContext(nc) as tc:
        with (
            tc.tile_pool(name="const", bufs=1) as constp,
            tc.tile_pool(name="hs", bufs=1) as hsp,
        ):
            # ACT table warmup: load the sigmoid/tanh spline set during DMAs
            warm = constp.tile([128, 128], dt.bfloat16)
            nc.gpsimd.memset(warm[:], 0.0)
            warm2 = constp.tile([128, 1], dt.float32)
            nc.scalar.activation(warm2[:], warm[:, :1], AF.Sigmoid)

            # ordered by first use; x loads go on gpsimd's DMA queue so the
            # first timesteps don't queue behind weights.
            # wih in per-(k, gate) 128KB chunks, alternating queues, in the
            # exact order the t=0 x matmuls consume them
            wih = constp.tile([128, 4, 3 * H], dt.bfloat16)
            wih8o = constp.tile([128, 4, H], dt.float8e4)
            q = 0
            for k in range(4):
                for n in range(3):
                    eng = nc.sync if q % 2 == 0 else nc.scalar
                    eng.dma_start(
                        wih[:, k, bass.ts(n, 512)], wih_d.ap()[:, k, bass.ts(n, 512)]
                    )
                    q += 1
                if k == 0:
                    # the o-gate fp8 weights are consumed at the end of every
                    # x block including t=0 — load them right after wih k0
                    nc.sync.dma_start(wih8o[:, 0:2, :], wih8o_d.ap()[:, 0:2, :])
                    nc.scalar.dma_start(wih8o[:, 2:4, :], wih8o_d.ap()[:, 2:4, :])
            bbc = constp.tile([128, G4], dt.bfloat16)
            nc.scalar.dma_start(bbc[:], bbc_d[:])
            ident = constp.tile([128, 128], dt.bfloat16)
            nc.gpsimd.dma_start(ident[:], ident_d[:])
            whh = constp.tile([128, 4, G4], dt.float8e4)
            for k in range(3):  # k=3 goes on the gpsimd queue in the t=0 body
                eng = nc.sync if k % 2 == 0 else nc.scalar
                eng.dma_start(whh[:, k, :], whh_d.ap()[:, k, :])
            wlin = constp.tile([128, 4, O], dt.bfloat16)
            # wlin DMA is issued inside the t=0 body, after xT[0]'s, so it
            # doesn't delay the first x matmuls on the gpsimd queue

            # h.T history (stored as 8*h.T): bf16 for the linear,
            # fp8 pair-sliceable for the DoubleRow recurrence
            hsT16 = hsp.tile([128, 4, NTOK], dt.bfloat16, name="hsT16")
            hsT8 = hsp.tile([128, 4, NTOK], dt.float8e4, name="hsT8")

            with (
                tc.tile_pool(name="xin", bufs=6) as xp,
                tc.tile_pool(name="xin8", bufs=6) as xp8,
                tc.tile_pool(name="gates", bufs=5, space="PSUM") as gps,
                tc.tile_pool(name="trps", bufs=1, space="PSUM") as trp,
                tc.tile_pool(name="linps", bufs=2, space="PSUM") as linps,
                tc.tile_pool(name="cell", bufs=3) as cp,
                tc.tile_pool(name="acts", bufs=8) as app,
                tc.tile_pool(name="linsb", bufs=6) as linsb,
            ):
                def emit_transpose(h_halves, t, need8=True):
                    tr = trp.tile([128, 512], dt.bfloat16, tag="tr")
                    for j in range(4):
                        nc.tensor.transpose(
                            tr[:, bass.ts(j, 128)],
                            h_halves[j // 2][:, bass.ts(j % 2, 128)],
                            ident[:],
                        )
                    # recurrence-critical fp8 copies go FIRST (the tile
                    # framework serializes readers of tr in program order)
                    # and on DVE, split in k-pair halves so the first
                    # DoubleRow matmul unblocks as early as possible
                    if need8:
                        # k-pair halves: copy8a (from h's first half) gates
                        # the j0 DoubleRow matmuls, so the recurrence starts
                        # while h's second half is still being transposed
                        nc.vector.tensor_copy(
                            hsT8[:, 0:2, bass.ts(t, 128)], tr[:, 0:256]
                        )
                        nc.vector.tensor_copy(
                            hsT8[:, 2:4, bass.ts(t, 128)], tr[:, 256:512]
                        )
                        # bf16 copy (feeds the linear, 5 steps behind) on ACT
                        nc.scalar.copy(hsT16[:, :, bass.ts(t, 128)], tr[:])
                    else:
                        nc.vector.tensor_copy(hsT16[:, :, bass.ts(t, 128)], tr[:])

                def emit_linear_mm(nch, m):
                    # outT[m] partial for token chunk nch (PE-idle filler)
                    ps = linps.tile(
                        [128, 512], dt.float32, tag="lps", name=f"lin{nch}_{m}"
                    )
                    for k in range(4):
                        nc.tensor.matmul(
                            ps[:], wlin[:, k, bass.ts(m, 128)],
                            hsT16[:, k, bass.ts(nch, 512)],
                            start=(k == 0), stop=(k == 3),
                        )
                    return ps

                def emit_linear_out(ps, nch, m):
                    # ACT copy deferred to the next step so it lands AFTER
                    # that step's hsT8 copy in the ACT FIFO (it must not
                    # delay the recurrence-critical copy)
                    ob = linsb.tile(
                        [128, 512], dt.float32, tag="ob", name=f"ob{nch}_{m}"
                    )
                    nc.scalar.copy(ob[:], ps[:])
                    eng = nc.sync if m % 2 == 0 else nc.scalar
                    eng.dma_start(oap[m, :, bass.ts(nch, 512)], ob[:])

                def emit_linear(nch, m):
                    emit_linear_out(emit_linear_mm(nch, m), nch, m)

                # PE warm-up during the initial weight DMAs: keeps the HAM
                # activity monitor from starting the real matmuls at the
                # throttled 1.2 GHz clock (zeros x zeros into scratch PSUM)
                wps = [
                    gps.tile([128, 512], dt.float32, tag="g", name=f"warm{i}")
                    for i in range(2)
                ]
                for w in range(8):
                    nc.tensor.matmul(
                        wps[w % 2][:, :128], warm[:], warm[:],
                        start=True, stop=True,
                    )

                c_prev = None
                h_prev = None
                lin_pending = None
                for t in range(T):
                    xt = xp.tile([128, 4, BL], dt.bfloat16, tag="xt")
                    nc.gpsimd.dma_start(xt[:], xap[t])
                    xt8 = xp8.tile([128, 4, BL], dt.float8e4, tag="xt8")
                    nc.gpsimd.dma_start(xt8[:], xap8[t])
                    if t == 0:
                        # third DMA channel for the recurrence weights' tail
                        nc.gpsimd.dma_start(whh[:, 3, :], whh_d.ap()[:, 3, :])
                    if t == 1:
                        nc.gpsimd.dma_start(wlin[:], wlin_d[:])

                    gt = [
                        gps.tile([128, 512], dt.float32, tag="g", name=f"g{t}_{n}")
                        for n in range(4)
                    ]
                    for k in range(4):
                        for n in range(3):
                            nc.tensor.matmul(
                                gt[n][:], xt[:, k, :], wih[:, k, bass.ts(n, 512)],
                                start=(k == 0), stop=(t == 0 and k == 3),
                            )
                    # o-gate x contribution in fp8 DoubleRow (x8 stationary)
                    for j in range(2):
                        nc.tensor.matmul(
                            gt[IO][:],
                            xt8[:, 2 * j:2 * j + 2, :],
                            wih8o[:, 2 * j:2 * j + 2, :],
                            start=(j == 0), stop=(t == 0 and j == 1),
                            perf_mode=DR,
                        )
                    if t > 0:
                        # transpose of h_{t-1} goes here: the x matmuls above
                        # cover step t-1's ACT/DVE chain latency
                        emit_transpose(h_prev, t - 1)
                        if lin_pending is not None:
                            emit_linear_out(*lin_pending)
                            lin_pending = None
                        if t >= 5:
                            # linear filler sits between the transposes and
                            # the recurrence matmuls: it covers the hsT8-copy
                            # wait so the PE never idles there
                            p = t - 5
                            lin_pending = (
                                emit_linear_mm(p // 4, p % 4), p // 4, p % 4
                            )
                        # fp8 DoubleRow recurrence: 2 k-pair matmuls per gate,
                        # gate-outer so each gate's PSUM tile completes early
                        for n in range(4):
                            for j in range(2):
                                nc.tensor.matmul(
                                    gt[n][:],
                                    hsT8[:, 2 * j:2 * j + 2, bass.ts(t - 1, 128)],
                                    whh[:, 2 * j:2 * j + 2, bass.ts(n, 512)],
                                    start=False, stop=(j == 1),
                                    perf_mode=DR,
                                )

                    # bias add (DVE, PSUM+SBUF -> SBUF); gates are x64, bias
                    # tile is 64*b; the /64 descale rides the ACT scale below
                    gb = [
                        app.tile([128, 512], dt.float32, tag="gb", name=f"gb{t}_{n}")
                        for n in range(4)
                    ]
                    acts = {}
                    for n, fn in ((IG, AF.Tanh), (II, AF.Sigmoid), (IF, AF.Sigmoid)):
                        nc.vector.tensor_add(gb[n][:], gt[n][:], bbc[:, bass.ts(n, 512)])
                        a = app.tile([128, 512], dt.bfloat16, tag="act", name=f"act{t}_{n}")
                        nc.scalar.activation(a[:], gb[n][:], fn, scale=1.0 / 64.0)
                        acts[n] = a
                    tg, i_s, f_s = acts[IG], acts[II], acts[IF]

                    c_new = cp.tile([128, 512], dt.bfloat16, tag="c")
                    if t == 0:
                        nc.vector.tensor_add(gb[IO][:], gt[IO][:], bbc[:, bass.ts(IO, 512)])
                        o_s = app.tile([128, 512], dt.bfloat16, tag="act", name=f"act{t}_o")
                        nc.scalar.activation(o_s[:], gb[IO][:], AF.Sigmoid, scale=1.0 / 64.0)
                        nc.vector.tensor_mul(c_new[:], i_s[:], tg[:])
                    else:
                        ig = cp.tile([128, 512], dt.bfloat16, tag="ig")
                        nc.vector.tensor_mul(ig[:], i_s[:], tg[:])
                        nc.vector.tensor_add(gb[IO][:], gt[IO][:], bbc[:, bass.ts(IO, 512)])
                        o_s = app.tile([128, 512], dt.bfloat16, tag="act", name=f"act{t}_o")
                        nc.scalar.activation(o_s[:], gb[IO][:], AF.Sigmoid, scale=1.0 / 64.0)
                        fc = cp.tile([128, 512], dt.bfloat16, tag="fc")
                        nc.vector.tensor_mul(fc[:], f_s[:], c_prev[:])
                        nc.vector.tensor_add(c_new[:], ig[:], fc[:])
                    c_prev = c_new

                    # tanh(c) -> h in halves: the first half unblocks the PE
                    # transposes earlier than a monolithic tail would
                    h_halves = []
                    for hh in range(2):
                        sl = bass.ts(hh, 256)
                        tch = app.tile(
                            [128, 256], dt.bfloat16, tag=f"tch{hh}", name=f"tch{t}_{hh}"
                        )
                        nc.scalar.activation(tch[:], c_new[:, sl], AF.Tanh)
                        hb = cp.tile(
                            [128, 256], dt.bfloat16, tag=f"h{hh}", name=f"h{t}_{hh}"
                        )
                        nc.vector.tensor_mul(hb[:], o_s[:, sl], tch[:])
                        h_halves.append(hb)
                    h_prev = h_halves

                emit_linear_out(*lin_pending)
                emit_linear(14, 3)
                # last token chunk split: tokens 60-62 (N=384) fill the PE
                # while step 63's ACT/DVE chain finishes; token 63 (N=128)
                # must wait for the final transpose
                # tail linear parts use the gates pool (free after step 63):
                # 4 tiles in flight, so the 16 matmuls stream without waiting
                # on per-m output copies
                for m in range(4):
                    ps = gps.tile([128, 384], dt.float32, tag="g", name=f"linA_{m}")
                    for k in range(4):
                        nc.tensor.matmul(
                            ps[:], wlin[:, k, bass.ts(m, 128)],
                            hsT16[:, k, 15 * 512:15 * 512 + 384],
                            start=(k == 0), stop=(k == 3),
                        )
                    ob = linsb.tile([128, 384], dt.float32, tag="ob", name=f"obA_{m}")
                    nc.vector.tensor_copy(ob[:], ps[:])
                    eng = nc.sync if m % 2 == 0 else nc.scalar
                    eng.dma_start(oap[m, :, 15 * 512:15 * 512 + 384], ob[:])
                emit_transpose(h_prev, T - 1, need8=False)
                for m in range(4):
                    ps = gps.tile([128, 128], dt.float32, tag="g", name=f"linB_{m}")
                    for k in range(4):
                        nc.tensor.matmul(
                            ps[:], wlin[:, k, bass.ts(m, 128)],
                            hsT16[:, k, 15 * 512 + 384:NTOK],
                            start=(k == 0), stop=(k == 3),
                        )
                    ob = linsb.tile([128, 128], dt.float32, tag="ob", name=f"obB_{m}")
                    nc.vector.tensor_copy(ob[:], ps[:])
                    eng = nc.sync if m % 2 == 0 else nc.scalar
                    eng.dma_start(oap[m, :, 15 * 512 + 384:NTOK], ob[:])


    nc.compile()
    return nc


def _get_program():
    global _PROGRAM
    if _PROGRAM is None:
        _PROGRAM = _build_program()
    return _PROGRAM


def _prep_core_inputs(xc, w_ih, w_hh, b, w_lin_half, backward):
    # xc: [BL, T, I] fp32 batch chunk
    if backward:
        xc = xc[:, ::-1, :]
    # [T, i_k(128) partitions, k(4), b(128)]
    xTf = np.ascontiguousarray(
        xc.transpose(1, 2, 0).reshape(T, 4, 128, BL).transpose(0, 2, 1, 3)
    )
    xT = xTf.astype(BF16)
    xT8 = xTf.astype(FP8)
    wp = 64.0 * w_ih[_PERM]  # [4H, I] in [g,i,f,o] order, pre-scaled
    wihT = np.ascontiguousarray(
        wp[: 3 * H].T.reshape(4, 128, 3 * H).transpose(1, 0, 2)
    ).astype(BF16)
    wih8o = np.ascontiguousarray(
        wp[3 * H:].T.reshape(4, 128, H).transpose(1, 0, 2)
    ).astype(FP8)
    whhT = np.ascontiguousarray(
        (64.0 * w_hh[_PERM]).T.reshape(4, 128, G4).transpose(1, 0, 2)
    ).astype(FP8)
    bbc = np.ascontiguousarray(
        np.broadcast_to((64.0 * b[_PERM])[None, :].astype(BF16), (128, G4))
    )
    wlinT = np.ascontiguousarray(
        w_lin_half.T.reshape(4, 128, O).transpose(1, 0, 2)
    ).astype(BF16)
    ident = np.eye(128, dtype=BF16)
    return dict(
        xT=xT, xT8=xT8, wihT=wihT, wih8o=wih8o, whhT=whhT, bbc=bbc,
        wlinT=wlinT, ident=ident,
    )


def kernel(x, w_ih_f, w_hh_f, b_f, w_ih_b, w_hh_b, b_b, w_lin, b_lin):
    global _LAST_RESULTS
    x = np.asarray(x, np.float32)
    w_ih_f = np.asarray(w_ih_f, np.float32)
    w_hh_f = np.asarray(w_hh_f, np.float32)
    b_f = np.asarray(b_f, np.float32)
    w_ih_b = np.asarray(w_ih_b, np.float32)
    w_hh_b = np.asarray(w_hh_b, np.float32)
    b_b = np.asarray(b_b, np.float32)
    w_lin = np.asarray(w_lin, np.float32)
    b_lin = np.asarray(b_lin, np.float32)

    nc = _get_program()
    in_maps = []
    for core in range(8):
        cidx = core % 4
        xc = x[cidx * BL:(cidx + 1) * BL]
        if core < 4:
            in_maps.append(
                _prep_core_inputs(xc, w_ih_f, w_hh_f, b_f, w_lin[:, :H], False)
            )
        else:
            in_maps.append(
                _prep_core_inputs(xc, w_ih_b, w_hh_b, b_b, w_lin[:, H:], True)
            )

    trace = bool(int(os.environ.get("LSTM_TRACE", "0")))
    tcores = os.environ.get("LSTM_TRACE_CORES", "")
    kwargs = {}
    if trace and tcores:
        kwargs["trace_cores"] = [int(c) for c in tcores.split(",")]
    res = run_bass_kernel_spmd(
        nc, in_maps, core_ids=list(range(8)), trace=trace, **kwargs
    )
    _LAST_RESULTS = res

    out = np.empty((B, T, O), np.float32)
    for cidx in range(4):
        pf = np.asarray(res.results[cidx]["outT"], np.float32)
        pb = np.asarray(res.results[cidx + 4]["outT"], np.float32)
        pf = pf.reshape(4, 128, T, BL).transpose(3, 2, 0, 1).reshape(BL, T, O)
        pb = pb.reshape(4, 128, T, BL).transpose(3, 2, 0, 1).reshape(BL, T, O)[:, ::-1]
        out[cidx * BL:(cidx + 1) * BL] = pf + pb + b_lin[None, None, :]
    return out


# revision 53
# speedup vs baseline: 1.1052x; 1.0344x over previous
"""Bidirectional LSTM on 8 Trainium2 NeuronCores (SPMD, Bass/Tile).

Problem:  x[512,64,512] -> BiLSTM(H=512) -> Linear(1024->512) -> out[512,64,512]

Sharding: batch 4-way x direction 2-way.
  core c   (c in 0..3): forward  LSTM, batch rows [c*128, (c+1)*128)
  core c+4            : backward LSTM, same rows (host passes x time-reversed)

v2: the recurrent GEMM runs in fp8e4m3 with DoubleRow (2 fp8 weights/PE cell,
K=256 per matmul -> 8 matmuls/step instead of 16). Numerics: h is ~4x smaller
than x in gate magnitude, so fp8 noise on the h-path lands at ~8e-3 rel err
(sim), vs 4e-2 if the x-path went fp8. Scale plumbing keeps one PSUM scale:
  gates_psum = x @ (64 W_ih)  +  h @ (64 W_hh)             [both x64]
  act = func(gates_psum * (1/64))                           [free ACT scale]
(64*W_hh in e4m3 is fine: |64 W_hh| <= 2.9, well inside the format; h is
stored unscaled — its sub-0.0156 values hit e4m3 subnormals but contribute
negligibly.) Cell state + elementwise chain run in bf16 (2x DVE modes); ACT
does the sigmoid/tanh with the /64 descale folded in.

Per-core device program (identical NEFF on all 8 cores, different data):
  for t in 0..63:
    g  = x_t @ W64_ih.T (bf16) + h_{t-1} @ 64W_hh.T (fp8 DoubleRow)   PE
    gg = tanh(g/64); i,f,o = sigmoid(g/64)        ACT, bf16 out
    c  = f*c + i*gg ; h = o*tanh(c)               DVE bf16
    hsT16[:,:,t*128:+128] = h.T (PE transpose + DVE copy), hsT8 = fp8 (ACT)
  outT = w_lin_half @ hsT16                       PE, streamed over 8192 tokens

Host: prepares transposed/scaled operands, gathers the 8 partial outputs,
adds forward+backward partials + b_lin in fp32, restores [B,T,O] layout.
"""

import os
import sys

import numpy as np
import ml_dtypes

sys.path.insert(0, "/opt/trn_rl_repo")

import concourse.bass as bass  # noqa: E402
import concourse.tile as tile  # noqa: E402
from concourse import bacc, mybir  # noqa: E402
from concourse.bass_utils import run_bass_kernel_spmd  # noqa: E402

BF16 = ml_dtypes.bfloat16
FP8 = ml_dtypes.float8_e4m3
B, T, I, H, O = 512, 64, 512, 512, 512
BL = 128        # batch rows per core
G4 = 4 * H      # 2048 gate rows
NTOK = T * BL   # 8192 tokens per core
AF = mybir.ActivationFunctionType
DR = mybir.MatmulPerfMode.DoubleRow

# our gate order [g, i, f, o]; pytorch rows are [i, f, g, o]
_PERM = np.concatenate([
    np.arange(2 * H, 3 * H),   # g (cell candidate)
    np.arange(0, H),           # i
    np.arange(H, 2 * H),       # f
    np.arange(3 * H, 4 * H),   # o
])
IG, II, IF, IO = 0, 1, 2, 3

_PROGRAM = None
_LAST_RESULTS = None


def _build_program():
    dt = mybir.dt
    nc = bacc.Bacc("TRN2", target_bir_lowering=False, debug=False)

    xT_d = nc.dram_tensor("xT", [T, 128, 4, BL], dt.bfloat16, kind="ExternalInput")
    xT8_d = nc.dram_tensor("xT8", [T, 128, 4, BL], dt.float8e4, kind="ExternalInput")
    # bf16 input weights for gates g,i,f only; the o-gate x-GEMM runs fp8
    # DoubleRow (sim: 1.27e-2 rel err, budget 2e-2)
    wih_d = nc.dram_tensor("wihT", [128, 4, 3 * H], dt.bfloat16, kind="ExternalInput")
    wih8o_d = nc.dram_tensor("wih8o", [128, 4, H], dt.float8e4, kind="ExternalInput")
    whh_d = nc.dram_tensor("whhT", [128, 4, G4], dt.float8e4, kind="ExternalInput")
    bbc_d = nc.dram_tensor("bbc", [128, G4], dt.bfloat16, kind="ExternalInput")
    wlin_d = nc.dram_tensor("wlinT", [128, 4, O], dt.bfloat16, kind="ExternalInput")
    ident_d = nc.dram_tensor("ident", [128, 128], dt.bfloat16, kind="ExternalInput")
    outT_d = nc.dram_tensor("outT", [4, 128, NTOK], dt.float32, kind="ExternalOutput")

    xap = xT_d.ap()
    xap8 = xT8_d.ap()
    oap = outT_d.ap()

    with tile.TileContext(nc) as tc:
        with (
            tc.tile_pool(name="const", bufs=1) as constp,
            tc.tile_pool(name="hs", bufs=1) as hsp,
        ):
            # ACT table warmup: load the sigmoid/tanh spline set during DMAs
            warm = constp.tile([128, 128], dt.bfloat16)
            nc.gpsimd.memset(warm[:], 0.0)
            warm2 = constp.tile([128, 1], dt.float32)
            nc.scalar.activation(warm2[:], warm[:, :1], AF.Sigmoid)

            # ordered by first use; x loads go on gpsimd's DMA queue so the
            # first timesteps don't queue behind weights.
            # wih in per-(k, gate) 128KB chunks, alternating queues, in the
            # exact order the t=0 x matmuls consume them
            wih = constp.tile([128, 4, 3 * H], dt.bfloat16)
            wih8o = constp.tile([128, 4, H], dt.float8e4)
            q = 0
            for k in range(4):
                for n in range(3):
                    eng = nc.sync if q % 2 == 0 else nc.scalar
                    eng.dma_start(
                        wih[:, k, bass.ts(n, 512)], wih_d.ap()[:, k, bass.ts(n, 512)]
                    )
                    q += 1
                if k == 0:
                    # the o-gate fp8 weights are consumed at the end of every
                    # x block including t=0 — load them right after wih k0
                    nc.sync.dma_start(wih8o[:, 0:2, :], wih8o_d.ap()[:, 0:2, :])
                    nc.scalar.dma_start(wih8o[:, 2:4, :], wih8o_d.ap()[:, 2:4, :])
            bbc = constp.tile([128, G4], dt.bfloat16)
            nc.scalar.dma_start(bbc[:], bbc_d[:])
            ident = constp.tile([128, 128], dt.bfloat16)
            nc.gpsimd.dma_start(ident[:], ident_d[:])
            whh = constp.tile([128, 4, G4], dt.float8e4)
            for k in range(3):  # k=3 goes on the gpsimd queue in the t=0 body
                eng = nc.sync if k % 2 == 0 else nc.scalar
                eng.dma_start(whh[:, k, :], whh_d.ap()[:, k, :])
            wlin = constp.tile([128, 4, O], dt.bfloat16)
            # wlin DMA is issued inside the t=0 body, after xT[0]'s, so it
            # doesn't delay the first x matmuls on the gpsimd queue

            # h.T history (stored as 8*h.T): bf16 for the linear,
            # fp8 pair-sliceable for the DoubleRow recurrence
            hsT16 = hsp.tile([128, 4, NTOK], dt.bfloat16, name="hsT16")
            hsT8 = hsp.tile([128, 4, NTOK], dt.float8e4, name="hsT8")

            with (
                tc.tile_pool(name="xin", bufs=6) as xp,
                tc.tile_pool(name="xin8", bufs=6) as xp8,
                tc.tile_pool(name="gates", bufs=5, space="PSUM") as gps,
                tc.tile_pool(name="trps", bufs=1, space="PSUM") as trp,
                tc.tile_pool(name="linps", bufs=2, space="PSUM") as linps,
                tc.tile_pool(name="cell", bufs=3) as cp,
                tc.tile_pool(name="acts", bufs=8) as app,
                tc.tile_pool(name="linsb", bufs=6) as linsb,
            ):
                def emit_transpose(h_halves, t, need8=True):
                    tr = trp.tile([128, 512], dt.bfloat16, tag="tr")
                    for j in range(4):
                        nc.tensor.transpose(
                            tr[:, bass.ts(j, 128)],
                            h_halves[j // 2][:, bass.ts(j % 2, 128)],
                            ident[:],
                        )
                    # recurrence-critical fp8 copies go FIRST (the tile
                    # framework serializes readers of tr in program order)
                    # and on DVE, split in k-pair halves so the first
                    # DoubleRow matmul unblocks as early as possible
                    if need8:
                        # single fp8 copy: both k-pairs become ready together,
                        # which keeps the scheduler's recurrence order compact
                        # (measured better than a split copy, twice)
                        nc.vector.tensor_copy(hsT8[:, :, bass.ts(t, 128)], tr[:])
                        # bf16 copy (feeds the linear, 5 steps behind) on ACT
                        nc.scalar.copy(hsT16[:, :, bass.ts(t, 128)], tr[:])
                    else:
                        nc.vector.tensor_copy(hsT16[:, :, bass.ts(t, 128)], tr[:])

                def emit_linear_mm(nch, m):
                    # outT[m] partial for token chunk nch (PE-idle filler)
                    ps = linps.tile(
                        [128, 512], dt.float32, tag="lps", name=f"lin{nch}_{m}"
                    )
                    for k in range(4):
                        nc.tensor.matmul(
                            ps[:], wlin[:, k, bass.ts(m, 128)],
                            hsT16[:, k, bass.ts(nch, 512)],
                            start=(k == 0), stop=(k == 3),
                        )
                    return ps

                def emit_linear_out(ps, nch, m):
                    # ACT copy deferred to the next step so it lands AFTER
                    # that step's hsT8 copy in the ACT FIFO (it must not
                    # delay the recurrence-critical copy)
                    ob = linsb.tile(
                        [128, 512], dt.float32, tag="ob", name=f"ob{nch}_{m}"
                    )
                    nc.scalar.copy(ob[:], ps[:])
                    eng = nc.sync if m % 2 == 0 else nc.scalar
                    eng.dma_start(oap[m, :, bass.ts(nch, 512)], ob[:])

                def emit_linear(nch, m):
                    emit_linear_out(emit_linear_mm(nch, m), nch, m)

                # PE warm-up during the initial weight DMAs: keeps the HAM
                # activity monitor from starting the real matmuls at the
                # throttled 1.2 GHz clock (zeros x zeros into scratch PSUM)
                wps = [
                    gps.tile([128, 512], dt.float32, tag="g", name=f"warm{i}")
                    for i in range(2)
                ]
                for w in range(8):
                    nc.tensor.matmul(
                        wps[w % 2][:, :128], warm[:], warm[:],
                        start=True, stop=True,
                    )

                c_prev = None
                h_prev = None
                lin_pending = None
                for t in range(T):
                    xt = xp.tile([128, 4, BL], dt.bfloat16, tag="xt")
                    nc.gpsimd.dma_start(xt[:], xap[t])
                    xt8 = xp8.tile([128, 4, BL], dt.float8e4, tag="xt8")
                    nc.gpsimd.dma_start(xt8[:], xap8[t])
                    if t == 0:
                        # third DMA channel for the recurrence weights' tail
                        nc.gpsimd.dma_start(whh[:, 3, :], whh_d.ap()[:, 3, :])
                    if t == 1:
                        nc.gpsimd.dma_start(wlin[:], wlin_d[:])

                    gt = [
                        gps.tile([128, 512], dt.float32, tag="g", name=f"g{t}_{n}")
                        for n in range(4)
                    ]
                    for k in range(4):
                        for n in range(3):
                            nc.tensor.matmul(
                                gt[n][:], xt[:, k, :], wih[:, k, bass.ts(n, 512)],
                                start=(k == 0), stop=(t == 0 and k == 3),
                            )
                    # o-gate x contribution in fp8 DoubleRow (x8 stationary)
                    for j in range(2):
                        nc.tensor.matmul(
                            gt[IO][:],
                            xt8[:, 2 * j:2 * j + 2, :],
                            wih8o[:, 2 * j:2 * j + 2, :],
                            start=(j == 0), stop=(t == 0 and j == 1),
                            perf_mode=DR,
                        )
                    if t > 0:
                        # transpose of h_{t-1} goes here: the x matmuls above
                        # cover step t-1's ACT/DVE chain latency
                        emit_transpose(h_prev, t - 1)
                        if lin_pending is not None:
                            emit_linear_out(*lin_pending)
                            lin_pending = None
                        if t >= 5:
                            # linear filler sits between the transposes and
                            # the recurrence matmuls: it covers the hsT8-copy
                            # wait so the PE never idles there
                            p = t - 5
                            lin_pending = (
                                emit_linear_mm(p // 4, p % 4), p // 4, p % 4
                            )
                        # fp8 DoubleRow recurrence: 2 k-pair matmuls per gate,
                        # gate-outer so each gate's PSUM tile completes early
                        for n in range(4):
                            for j in range(2):
                                nc.tensor.matmul(
                                    gt[n][:],
                                    hsT8[:, 2 * j:2 * j + 2, bass.ts(t - 1, 128)],
                                    whh[:, 2 * j:2 * j + 2, bass.ts(n, 512)],
                                    start=False, stop=(j == 1),
                                    perf_mode=DR,
                                )

                    # bias add (DVE, PSUM+SBUF -> SBUF); gates are x64, bias
                    # tile is 64*b; the /64 descale rides the ACT scale below
                    gb = [
                        app.tile([128, 512], dt.float32, tag="gb", name=f"gb{t}_{n}")
                        for n in range(4)
                    ]
                    acts = {}
                    for n, fn in ((IG, AF.Tanh), (II, AF.Sigmoid), (IF, AF.Sigmoid)):
                        nc.vector.tensor_add(gb[n][:], gt[n][:], bbc[:, bass.ts(n, 512)])
                        a = app.tile([128, 512], dt.bfloat16, tag="act", name=f"act{t}_{n}")
                        nc.scalar.activation(a[:], gb[n][:], fn, scale=1.0 / 64.0)
                        acts[n] = a
                    tg, i_s, f_s = acts[IG], acts[II], acts[IF]

                    c_new = cp.tile([128, 512], dt.bfloat16, tag="c")
                    if t == 0:
                        nc.vector.tensor_add(gb[IO][:], gt[IO][:], bbc[:, bass.ts(IO, 512)])
                        o_s = app.tile([128, 512], dt.bfloat16, tag="act", name=f"act{t}_o")
                        nc.scalar.activation(o_s[:], gb[IO][:], AF.Sigmoid, scale=1.0 / 64.0)
                        nc.vector.tensor_mul(c_new[:], i_s[:], tg[:])
                    else:
                        ig = cp.tile([128, 512], dt.bfloat16, tag="ig")
                        nc.vector.tensor_mul(ig[:], i_s[:], tg[:])
                        nc.vector.tensor_add(gb[IO][:], gt[IO][:], bbc[:, bass.ts(IO, 512)])
                        o_s = app.tile([128, 512], dt.bfloat16, tag="act", name=f"act{t}_o")
                        nc.scalar.activation(o_s[:], gb[IO][:], AF.Sigmoid, scale=1.0 / 64.0)
                        fc = cp.tile([128, 512], dt.bfloat16, tag="fc")
                        nc.vector.tensor_mul(fc[:], f_s[:], c_prev[:])
                        nc.vector.tensor_add(c_new[:], ig[:], fc[:])
                    c_prev = c_new

                    # tanh(c) -> h in halves: the first half unblocks the PE
                    # transposes earlier than a monolithic tail would
                    h_halves = []
                    for hh in range(2):
                        sl = bass.ts(hh, 256)
                        tch = app.tile(
                            [128, 256], dt.bfloat16, tag=f"tch{hh}", name=f"tch{t}_{hh}"
                        )
                        nc.scalar.activation(tch[:], c_new[:, sl], AF.Tanh)
                        hb = cp.tile(
                            [128, 256], dt.bfloat16, tag=f"h{hh}", name=f"h{t}_{hh}"
                        )
                        nc.vector.tensor_mul(hb[:], o_s[:, sl], tch[:])
                        h_halves.append(hb)
                    h_prev = h_halves

                emit_linear_out(*lin_pending)
                emit_linear(14, 3)
                # last token chunk split: tokens 60-62 (N=384) fill the PE
                # while step 63's ACT/DVE chain finishes; token 63 (N=128)
                # must wait for the final transpose
                # tail linear parts use the gates pool (free after step 63):
                # 4 tiles in flight, so the 16 matmuls stream without waiting
                # on per-m output copies
                for m in range(4):
                    ps = gps.tile([128, 384], dt.float32, tag="g", name=f"linA_{m}")
                    for k in range(4):
                        nc.tensor.matmul(
                            ps[:], wlin[:, k, bass.ts(m, 128)],
                            hsT16[:, k, 15 * 512:15 * 512 + 384],
                            start=(k == 0), stop=(k == 3),
                        )
                    ob = linsb.tile([128, 384], dt.float32, tag="ob", name=f"obA_{m}")
                    nc.vector.tensor_copy(ob[:], ps[:])
                    eng = nc.sync if m % 2 == 0 else nc.scalar
                    eng.dma_start(oap[m, :, 15 * 512:15 * 512 + 384], ob[:])
                emit_transpose(h_prev, T - 1, need8=False)
                for m in range(4):
                    ps = gps.tile([128, 128], dt.float32, tag="g", name=f"linB_{m}")
                    for k in range(4):
                        nc.tensor.matmul(
                            ps[:], wlin[:, k, bass.ts(m, 128)],
                            hsT16[:, k, 15 * 512 + 384:NTOK],
                            start=(k == 0), stop=(k == 3),
                        )
                    ob = linsb.tile([128, 128], dt.float32, tag="ob", name=f"obB_{m}")
                    nc.vector.tensor_copy(ob[:], ps[:])
                    eng = nc.sync if m % 2 == 0 else nc.scalar
                    eng.dma_start(oap[m, :, 15 * 512 + 384:NTOK], ob[:])


    nc.compile()
    return nc


def _get_program():
    global _PROGRAM
    if _PROGRAM is None:
        _PROGRAM = _build_program()
    return _PROGRAM


def _prep_core_inputs(xc, w_ih, w_hh, b, w_lin_half, backward):
    # xc: [BL, T, I] fp32 batch chunk
    if backward:
        xc = xc[:, ::-1, :]
    # [T, i_k(128) partitions, k(4), b(128)]
    xTf = np.ascontiguousarray(
        xc.transpose(1, 2, 0).reshape(T, 4, 128, BL).transpose(0, 2, 1, 3)
    )
    xT = xTf.astype(BF16)
    xT8 = xTf.astype(FP8)
    wp = 64.0 * w_ih[_PERM]  # [4H, I] in [g,i,f,o] order, pre-scaled
    wihT = np.ascontiguousarray(
        wp[: 3 * H].T.reshape(4, 128, 3 * H).transpose(1, 0, 2)
    ).astype(BF16)
    wih8o = np.ascontiguousarray(
        wp[3 * H:].T.reshape(4, 128, H).transpose(1, 0, 2)
    ).astype(FP8)
    whhT = np.ascontiguousarray(
        (64.0 * w_hh[_PERM]).T.reshape(4, 128, G4).transpose(1, 0, 2)
    ).astype(FP8)
    bbc = np.ascontiguousarray(
        np.broadcast_to((64.0 * b[_PERM])[None, :].astype(BF16), (128, G4))
    )
    wlinT = np.ascontiguousarray(
        w_lin_half.T.reshape(4, 128, O).transpose(1, 0, 2)
    ).astype(BF16)
    ident = np.eye(128, dtype=BF16)
    return dict(
        xT=xT, xT8=xT8, wihT=wihT, wih8o=wih8o, whhT=whhT, bbc=bbc,
        wlinT=wlinT, ident=ident,
    )


def kernel(x, w_ih_f, w_hh_f, b_f, w_ih_b, w_hh_b, b_b, w_lin, b_lin):
    global _LAST_RESULTS
    x = np.asarray(x, np.float32)
    w_ih_f = np.asarray(w_ih_f, np.float32)
    w_hh_f = np.asarray(w_hh_f, np.float32)
    b_f = np.asarray(b_f, np.float32)
    w_ih_b = np.asarray(w_ih_b, np.float32)
    w_hh_b = np.asarray(w_hh_b, np.float32)
    b_b = np.asarray(b_b, np.float32)
    w_lin = np.asarray(w_lin, np.float32)
    b_lin = np.asarray(b_lin, np.float32)

    nc = _get_program()
    in_maps = []
    for core in range(8):
        cidx = core % 4
        xc = x[cidx * BL:(cidx + 1) * BL]
        if core < 4:
            in_maps.append(
                _prep_core_inputs(xc, w_ih_f, w_hh_f, b_f, w_lin[:, :H], False)
            )
        else:
            in_maps.append(
                _prep_core_inputs(xc, w_ih_b, w_hh_b, b_b, w_lin[:, H:], True)
            )

    trace = bool(int(os.environ.get("LSTM_TRACE", "0")))
    tcores = os.environ.get("LSTM_TRACE_CORES", "")
    kwargs = {}
    if trace and tcores:
        kwargs["trace_cores"] = [int(c) for c in tcores.split(",")]
    res = run_bass_kernel_spmd(
        nc, in_maps, core_ids=list(range(8)), trace=trace, **kwargs
    )
    _LAST_RESULTS = res

    out = np.empty((B, T, O), np.float32)
    for cidx in range(4):
        pf = np.asarray(res.results[cidx]["outT"], np.float32)
        pb = np.asarray(res.results[cidx + 4]["outT"], np.float32)
        pf = pf.reshape(4, 128, T, BL).transpose(3, 2, 0, 1).reshape(BL, T, O)
        pb = pb.reshape(4, 128, T, BL).transpose(3, 2, 0, 1).reshape(BL, T, O)[:, ::-1]
        out[cidx * BL:(cidx + 1) * BL] = pf + pb + b_lin[None, None, :]
    return out
